# revision 17
# baseline (speedup 1.0000x reference)
"""LoRA generator kernel for Trainium2, sharded over 8 NeuronCores by layer.

Reference computation (see problem):
  pe = (condition @ W_proj + b_proj)                        (B=2, 224, 512)
  A  = (gelu(pe@WA1+bA1) @ WA2 + bA2) -> (B, L, 7, 16, 64)
  Bm = (gelu(pe@WB1+bB1) @ WB2 + bB2) -> (B, L, 7, 64, 16)
  out per (b, layer): concat over t of [tile_cols(A)*scA (16 x in_d),
                                        tile_rows(B)*scB (out_d x 16)]

Each core handles 4 layers (28 of the 224 projections). The big costs are
streaming its W_proj slice (11MB in bf16) and writing its 36.8MB output
slice. Pipeline: 4 rounds, one layer each — round r loads layer r's W_proj
columns (one 2.75MB DMA), computes pe, decodes, and drains the layer's
output while round r+1 loads.

Output path is built for large DMA descriptors:
  A pieces: decoder rows are scattered to partition (rank*8+slot), the
    64-float base chunk is expanded to 4096 floats by 6 DVE doubling
    copies, and each piece is written with 16KB descriptors (one per
    rank-row; in_d=11008 uses a step-0 repeat plus tail).
  B pieces: each piece is one 4KB block repeated out_d/64 times; the block
    is placed on an engine-balanced partition with 4 copies materialized
    (16KB), then written with 16KB descriptors using a step-0 repeat dim.
"""
import sys

sys.path.insert(0, "/opt/trn_rl_repo")

import numpy as np
import ml_dtypes

import concourse.bass as bass
import concourse.bacc as bacc
import concourse.mybir as mybir
import concourse.tile as tile
from concourse.bass_utils import run_bass_kernel_spmd

F32 = mybir.dt.float32
BF16 = mybir.dt.bfloat16
ACT_FN = mybir.ActivationFunctionType.Gelu  # sim override hook
SCATTER_ENG = lambda nc: nc.sync  # probe hook
NPBF16 = ml_dtypes.bfloat16

NCORES = 8
NUM_LAYERS = 32
RANK = 16
PED = 512
EMB = 384
T = 7
L = NUM_LAYERS // NCORES          # 4 layers per core
LT = L * T                        # 28 projections per core
ROWS = 2 * LT                     # 56 rows (b, l, t); row = (l*7+t)*2 + b
WP_COLS = LT * PED                # 14336
RPL = 2 * T                       # 14 rows per layer

IN_DS = [4096, 4096, 4096, 4096, 4096, 4096, 11008]
OUT_DS = [4096, 1024, 1024, 4096, 11008, 11008, 4096]
A_SIZES = [16 * d for d in IN_DS]
B_SIZES = [16 * d for d in OUT_DS]
LAYER_SIZE = sum(A_SIZES) + sum(B_SIZES)   # 1150976
OFF_A = []
OFF_B = []
_o = 0
for _t in range(T):
    OFF_A.append(_o)
    _o += A_SIZES[_t]
    OFF_B.append(_o)
    _o += B_SIZES[_t]
OUT_SZ = 2 * L * LAYER_SIZE

RCOLS = T * PED                   # 3584 W_proj columns per round (1 layer)

PB_L = [0, 32, 64, 64]           # partition base per layer (engine ops need 0/32/64)
ACOL = [0, 0, 0, 1024]           # oa column offset per layer
BCOL = [0, 0, 0, 1024]           # ob / scaled-bias column offset per layer

# slot -> (t, b) within a group; even groups hold rows 0-6 of the layer,
# odd groups rows 7-13, where row = 2*t + b.
SLOT_TB = [
    [(0, 0), (0, 1), (1, 0), (1, 1), (2, 0), (2, 1), (3, 0)],
    [(3, 1), (4, 0), (4, 1), (5, 0), (5, 1), (6, 0), (6, 1)],
]

# B piece placement: partition for (t, b) chosen so the 14 pieces of a layer
# land on 14 distinct SDMA engines (p<64 -> even engines, p>=64 -> odd).
B_PART = {
    (0, 0): 0, (0, 1): 4,
    (1, 0): 8, (1, 1): 20,
    (2, 0): 12, (2, 1): 16,
    (3, 0): 24, (3, 1): 28,
    (4, 0): 64, (4, 1): 68,
    (5, 0): 72, (5, 1): 76,
    (6, 0): 80, (6, 1): 84,
}
NB4 = [d // 256 for d in OUT_DS]   # 16KB-descriptor repeats per piece


def _gbase(g):
    """First partition of row-group g (7 rows each, g = 2*l + parity)."""
    return PB_L[g // 2] + 7 * (g % 2)


def _build_nc():
    nc = bacc.Bacc(None, target_bir_lowering=False, debug=False)

    cond = nc.declare_dram_parameter("cond", [128, 6], BF16, isOutput=False)
    wp = nc.declare_dram_parameter("wp", [EMB, WP_COLS], BF16, isOutput=False)
    bpt = nc.declare_dram_parameter("bpt", [128, 4 * LT], F32, isOutput=False)
    wa1 = nc.declare_dram_parameter("wa1", [128, 1024], BF16, isOutput=False)
    wb1 = nc.declare_dram_parameter("wb1", [128, 1024], BF16, isOutput=False)
    wa2 = nc.declare_dram_parameter("wa2", [128, 2048], BF16, isOutput=False)
    wb2 = nc.declare_dram_parameter("wb2", [128, 2048], BF16, isOutput=False)
    ba1 = nc.declare_dram_parameter("ba1", [128, 2], F32, isOutput=False)
    bb1 = nc.declare_dram_parameter("bb1", [128, 2], F32, isOutput=False)
    sca = nc.declare_dram_parameter("sca", [128, ROWS], BF16, isOutput=False)
    scb = nc.declare_dram_parameter("scb", [128, ROWS], BF16, isOutput=False)
    sba2 = nc.declare_dram_parameter("sba2", [128, 2048], F32, isOutput=False)
    sbb2 = nc.declare_dram_parameter("sbb2", [128, 2048], F32, isOutput=False)
    ident = nc.declare_dram_parameter("ident", [128, 2], F32, isOutput=False)
    out = nc.declare_dram_parameter("out", [OUT_SZ], F32, isOutput=True)

    with tile.TileContext(nc) as tc:
        with (
            tc.tile_pool(name="const", bufs=1) as cpool,
            tc.tile_pool(name="wp", bufs=2) as wpool,
            tc.tile_pool(name="work", bufs=1) as wkpool,
            tc.tile_pool(name="pe2", bufs=2) as pe2pool,
            tc.tile_pool(name="ps", bufs=1, space="PSUM") as ps,
        ):
            cond_sb = cpool.tile([128, 6], BF16)
            nc.gpsimd.dma_start(cond_sb[:], cond[:])
            bpt_sb = cpool.tile([128, 4 * LT], F32)
            nc.sync.dma_start(bpt_sb[:], bpt[:])
            wa1_sb = cpool.tile([128, 1024], BF16)
            nc.sync.dma_start(wa1_sb[:], wa1[:])
            wb1_sb = cpool.tile([128, 1024], BF16)
            nc.sync.dma_start(wb1_sb[:], wb1[:])
            wa2_sb = cpool.tile([128, 2048], BF16)
            nc.sync.dma_start(wa2_sb[:], wa2[:])
            wb2_sb = cpool.tile([128, 2048], BF16)
            nc.sync.dma_start(wb2_sb[:], wb2[:])
            ba1_sb = cpool.tile([128, 2], F32)
            nc.sync.dma_start(ba1_sb[:], ba1[:])
            bb1_sb = cpool.tile([128, 2], F32)
            nc.sync.dma_start(bb1_sb[:], bb1[:])
            sca_sb = cpool.tile([128, ROWS], BF16)
            nc.sync.dma_start(sca_sb[:], sca[:])
            scb_sb = cpool.tile([128, ROWS], BF16)
            nc.sync.dma_start(scb_sb[:], scb[:])
            sba2_sb = cpool.tile([128, 2048], F32)
            nc.scalar.dma_start(sba2_sb[:], sba2[:])
            sbb2_sb = cpool.tile([128, 2048], F32)
            nc.scalar.dma_start(sbb2_sb[:], sbb2[:])
            ident_sb = cpool.tile([128, 2], F32)
            nc.sync.dma_start(ident_sb[:], ident[:])

            # long-lived work tiles
            pe_sb = [
                wkpool.tile([128, ROWS], BF16, tag=f"pe_sb{mc}", name=f"pe_sb{mc}")
                for mc in range(4)
            ]
            oa = wkpool.tile([128, 2048], F32)     # decoder A out
            ob_sb = wkpool.tile([128, 2048], F32)  # decoder B out
            pa = oa[:, :].ap[0][0]
            pob = ob_sb[:, :].ap[0][0]
            oa_t = oa[:, :].tensor
            ob_t = ob_sb[:, :].tensor
            # expansion buffers, rotated manually (layer l+k reuses after
            # the piece DMAs of layer l drained; Tile tracks the WAR deps)
            aexp_bufs = [
                wkpool.tile([128, 4096], F32, tag=f"aexpb{i}", name=f"aexpb{i}")
                for i in range(3)
            ]
            bexp_bufs = [
                wkpool.tile([128, 4096], F32, tag=f"bexpb{i}", name=f"bexpb{i}")
                for i in range(2)
            ]

            def decode_layer(l):
                """Decoder MLPs + expansion + piece DMAs for layer l."""
                c0 = RPL * l              # first row / pe_sb column of the layer
                pb = PB_L[l]              # partition base (0/32/64)
                acol, bcol = ACOL[l], BCOL[l]
                for dec, (w1_sb, b1_sb, w2_sb, sc_sb, sb2_sb) in enumerate(
                    [
                        (wa1_sb, ba1_sb, wa2_sb, sca_sb, sba2_sb),
                        (wb1_sb, bb1_sb, wb2_sb, scb_sb, sbb2_sb),
                    ]
                ):
                    h_sb = []
                    for mc in range(2):
                        hp = ps.tile([128, RPL], F32, tag=f"h{mc}", name=f"hp{mc}")
                        for kc in range(4):
                            nc.tensor.matmul(
                                hp[:],
                                w1_sb[:, kc * 256 + mc * 128 : kc * 256 + (mc + 1) * 128],
                                pe_sb[kc][:, c0 : c0 + RPL],
                                start=(kc == 0),
                                stop=(kc == 3),
                            )
                        hs = wkpool.tile(
                            [128, RPL], BF16, tag=f"h_sb{dec}{mc}", name=f"hs{dec}{mc}"
                        )
                        nc.scalar.activation(
                            hs[:], hp[:], ACT_FN,
                            bias=b1_sb[:, mc : mc + 1],
                        )
                        nc.vector.tensor_mul(hs[:], hs[:], sc_sb[:, c0 : c0 + RPL])
                        h_sb.append(hs)
                    for nh in range(2):
                        op = ps.tile([128, 512], F32, tag=f"o{nh}", name=f"op{nh}")
                        for kc in range(2):
                            nc.tensor.matmul(
                                op[pb : pb + RPL, :],
                                h_sb[kc][:],
                                w2_sb[:, kc * 1024 + nh * 512 : kc * 1024 + (nh + 1) * 512],
                                start=(kc == 0),
                                stop=(kc == 1),
                            )
                        tgt = oa if dec == 0 else ob_sb
                        coff = acol if dec == 0 else bcol
                        nc.vector.tensor_add(
                            tgt[pb : pb + RPL, coff + nh * 512 : coff + (nh + 1) * 512],
                            op[pb : pb + RPL, :],
                            sb2_sb[pb : pb + RPL, bcol + nh * 512 : bcol + (nh + 1) * 512],
                        )

                # ---- A pieces ----
                for g in (2 * l, 2 * l + 1):
                    gb = _gbase(g)
                    aexp = aexp_bufs[g % 3]
                    aexp_t = aexp[:, :].tensor
                    pax = aexp[:, :].ap[0][0]
                    # partitions p%8==7 are never scattered to; zero the seed
                    # region so the doubling copies read initialized data
                    nc.vector.memset(aexp[:, 0:64], 0)
                    # scatter: aexp[8r+slot, 0:64] = oa[gb+slot, acol+64r : +64]
                    # (one DMA per slot — SBUF APs allow only one
                    # partition-crossing dim, the first)
                    for s in range(7):
                        src = oa[gb + s : gb + s + 1, acol : acol + 1024]
                        dst = bass.AP(
                            aexp_t, s * pax, [[8 * pax, 16], [1, 64]]
                        )
                        SCATTER_ENG(nc).dma_start(dst, src)
                    # expand 64 -> 4096 by doubling (DVE)
                    w = 64
                    while w < 4096:
                        nc.vector.tensor_copy(
                            aexp[:, w : 2 * w], aexp[:, 0:w]
                        )
                        w *= 2
                    # piece DMAs: 16KB descriptors, one per rank-row
                    for s in range(7):
                        t, b = SLOT_TB[g % 2][s]
                        in_d = IN_DS[t]
                        base = (b * L + l) * LAYER_SIZE + OFF_A[t]
                        if in_d == 4096:
                            dstp = bass.AP(out, base, [[4096, 16], [1, 4096]])
                            srcp = bass.AP(
                                aexp_t, s * pax, [[8 * pax, 16], [1, 4096]]
                            )
                            nc.sync.dma_start(dstp, srcp)
                        else:  # 11008 = 2*4096 + 2816
                            dstp = bass.AP(
                                out, base, [[in_d, 16], [4096, 2], [1, 4096]]
                            )
                            srcp = bass.AP(
                                aexp_t, s * pax, [[8 * pax, 16], [0, 2], [1, 4096]]
                            )
                            nc.sync.dma_start(dstp, srcp)
                            dstp = bass.AP(
                                out, base + 8192, [[in_d, 16], [1, 2816]]
                            )
                            srcp = bass.AP(
                                aexp_t, s * pax, [[8 * pax, 16], [1, 2816]]
                            )
                            nc.sync.dma_start(dstp, srcp)

                # ---- B pieces ----
                bexp = bexp_bufs[l % 2]
                bexp_t = bexp[:, :].tensor
                pbx = bexp[:, :].ap[0][0]
                for t in range(T):
                    p0 = B_PART[(t, 0)]
                    dp = B_PART[(t, 1)] - p0
                    # spread + 4x replicate: bexp[p(t,b), 0:4096] = ob row x4
                    src = bass.AP(
                        ob_t,
                        (pb + 2 * t) * pob + bcol,
                        [[pob, 2], [0, 4], [1, 1024]],
                    )
                    dst = bass.AP(
                        bexp_t, p0 * pbx, [[dp * pbx, 2], [1024, 4], [1, 1024]]
                    )
                    nc.scalar.dma_start(dst, src)
                for t in range(T):
                    p0 = B_PART[(t, 0)]
                    dp = B_PART[(t, 1)] - p0
                    nb4 = NB4[t]
                    dstp = bass.AP(
                        out,
                        l * LAYER_SIZE + OFF_B[t],
                        [[L * LAYER_SIZE, 2], [4096, nb4], [1, 4096]],
                    )
                    srcp = bass.AP(
                        bexp_t, p0 * pbx, [[dp * pbx, 2], [0, nb4], [1, 4096]]
                    )
                    nc.scalar.dma_start(dstp, srcp)

            # ---- main pipeline: one layer per round ----
            for rd in range(L):
                wp_t = wpool.tile([128, 3 * RCOLS], BF16, tag="wp", name=f"wp{rd}")
                pwt = wp_t[:, :].ap[0][0]
                wp_src = bass.AP(
                    wp, rd * RCOLS, [[WP_COLS, 128], [128 * WP_COLS, 3], [1, RCOLS]]
                )
                wp_dst = bass.AP(
                    wp_t[:, :].tensor, 0, [[pwt, 128], [RCOLS, 3], [1, RCOLS]]
                )
                nc.gpsimd.dma_start(wp_dst, wp_src)
                pe2_sb = pe2pool.tile([2, RCOLS], F32, tag="pe2sb", name="pe2_sb")
                for ltl in range(T):
                    p2 = ps.tile([2, PED], F32, tag=f"p2{ltl % 2}", name="pe2_ps")
                    for kc in range(3):
                        nc.tensor.matmul(
                            p2[:],
                            cond_sb[:, kc * 2 : kc * 2 + 2],
                            wp_t[:, kc * RCOLS + ltl * PED : kc * RCOLS + (ltl + 1) * PED],
                            start=(kc == 0),
                            stop=(kc == 2),
                        )
                    nc.vector.tensor_copy(pe2_sb[:, ltl * PED : (ltl + 1) * PED], p2[:])
                for ltl in range(T):
                    lt = rd * T + ltl
                    for mc in range(4):
                        tr = ps.tile([128, 2], F32, tag=f"tr{mc % 2}", name="tr_ps")
                        nc.tensor.transpose(
                            tr[:],
                            pe2_sb[:, ltl * PED + mc * 128 : ltl * PED + (mc + 1) * 128],
                            ident_sb[0:2, 0:2],
                        )
                        # pe_T with b_proj bias (per-partition, same for both b)
                        nc.vector.tensor_scalar_add(
                            pe_sb[mc][:, 2 * lt : 2 * lt + 2],
                            tr[:],
                            bpt_sb[:, mc * LT + lt : mc * LT + lt + 1],
                        )
                decode_layer(rd)

    nc.finalize()
    return nc


_NC = None


def _get_nc():
    global _NC
    if _NC is None:
        _NC = _build_nc()
    return _NC


def _marshal(inputs):
    """Build the per-core input maps from full inputs."""
    condition = np.asarray(inputs["condition"], np.float32)
    W_proj = np.asarray(inputs["W_proj"], np.float32)
    b_proj = np.asarray(inputs["b_proj"], np.float32)
    WA1 = np.asarray(inputs["WA1"], np.float32)
    bA1 = np.asarray(inputs["bA1"], np.float32)
    WA2 = np.asarray(inputs["WA2"], np.float32)
    bA2 = np.asarray(inputs["bA2"], np.float32)
    WB1 = np.asarray(inputs["WB1"], np.float32)
    bB1 = np.asarray(inputs["bB1"], np.float32)
    WB2 = np.asarray(inputs["WB2"], np.float32)
    bB2 = np.asarray(inputs["bB2"], np.float32)
    scales = np.asarray(inputs["scales"], np.float32)

    cond_arr = np.zeros((128, 6), np.float32)
    for kc in range(3):
        cond_arr[:, kc * 2 : kc * 2 + 2] = condition[:, kc * 128 : (kc + 1) * 128].T
    cond_arr = cond_arr.astype(NPBF16)
    wa1_arr = np.zeros((128, 1024), np.float32)
    wb1_arr = np.zeros((128, 1024), np.float32)
    for kc in range(4):
        wa1_arr[:, kc * 256 : (kc + 1) * 256] = WA1[kc * 128 : (kc + 1) * 128, :]
        wb1_arr[:, kc * 256 : (kc + 1) * 256] = WB1[kc * 128 : (kc + 1) * 128, :]
    wa2_arr = np.zeros((128, 2048), np.float32)
    wb2_arr = np.zeros((128, 2048), np.float32)
    for kc in range(2):
        wa2_arr[:, kc * 1024 : (kc + 1) * 1024] = WA2[kc * 128 : (kc + 1) * 128, :]
        wb2_arr[:, kc * 1024 : (kc + 1) * 1024] = WB2[kc * 128 : (kc + 1) * 128, :]
    wa1_arr = wa1_arr.astype(NPBF16)
    wb1_arr = wb1_arr.astype(NPBF16)
    wa2_arr = wa2_arr.astype(NPBF16)
    wb2_arr = wb2_arr.astype(NPBF16)
    ba1_arr = np.ascontiguousarray(bA1.reshape(2, 128).T)
    bb1_arr = np.ascontiguousarray(bB1.reshape(2, 128).T)
    ident_arr = np.zeros((128, 2), np.float32)
    ident_arr[0, 0] = 1.0
    ident_arr[1, 1] = 1.0

    in_maps = []
    for c in range(NCORES):
        lt0 = c * LT
        wp_c = np.ascontiguousarray(
            W_proj[:, lt0 * PED : (lt0 + LT) * PED]
        ).astype(NPBF16)
        bp_c = b_proj[lt0 * PED : (lt0 + LT) * PED].reshape(LT, 4, 128)
        bpt_arr = np.zeros((128, 4 * LT), np.float32)
        for lt in range(LT):
            for mc in range(4):
                bpt_arr[:, mc * LT + lt] = bp_c[lt, mc, :]
        sca_row = np.zeros(ROWS, np.float32)
        scb_row = np.zeros(ROWS, np.float32)
        for row in range(ROWS):
            lt = row // 2
            sca_row[row] = scales[lt0 + lt, 0]
            scb_row[row] = scales[lt0 + lt, 1]
        sca_arr = np.broadcast_to(sca_row[None, :], (128, ROWS)).astype(NPBF16)
        scb_arr = np.broadcast_to(scb_row[None, :], (128, ROWS)).astype(NPBF16)
        sba2_arr = np.zeros((128, 2048), np.float32)
        sbb2_arr = np.zeros((128, 2048), np.float32)
        for row in range(ROWS):
            l = row // RPL
            p = PB_L[l] + (row % RPL)
            blk = BCOL[l]
            sba2_arr[p, blk : blk + 1024] = sca_row[row] * bA2
            sbb2_arr[p, blk : blk + 1024] = scb_row[row] * bB2
        in_maps.append(
            {
                "cond": cond_arr,
                "wp": wp_c,
                "bpt": bpt_arr,
                "wa1": wa1_arr,
                "wb1": wb1_arr,
                "wa2": wa2_arr,
                "wb2": wb2_arr,
                "ba1": ba1_arr,
                "bb1": bb1_arr,
                "sca": sca_arr,
                "scb": scb_arr,
                "sba2": sba2_arr,
                "sbb2": sbb2_arr,
                "ident": ident_arr,
            }
        )
    return in_maps


def _ensure_ntff_hook():
    """Register the axon NTFF profile hook if the boot didn't (module was
    missing at boot time)."""
    import types

    ah = sys.modules.get("antenv.axon_hooks")
    if ah is None:
        ah = types.ModuleType("antenv.axon_hooks")
        ah._hook = None

        def _set(h, _m=ah):
            _m._hook = h

        def _get(_m=ah):
            return _m._hook

        ah.set_axon_ntff_profile_hook = _set
        ah.get_axon_ntff_profile_hook = _get
        sys.modules["antenv.axon_hooks"] = ah
        import antenv

        antenv.axon_hooks = ah
    if ah.get_axon_ntff_profile_hook() is None:
        if "/root/.axon_site" not in sys.path:
            sys.path.insert(0, "/root/.axon_site")
        from trn_agent_boot.trn_boot import _ntff_profile_via_ctypes

        hook = _ntff_profile_via_ctypes("/opt/axon/libaxon_pjrt.so")
        if hook is not None:
            ah.set_axon_ntff_profile_hook(hook)


def _run(inputs, trace=False):
    if trace:
        _ensure_ntff_hook()
    nc = _get_nc()
    in_maps = _marshal(inputs)
    res = run_bass_kernel_spmd(nc, in_maps, list(range(NCORES)), trace=trace)
    full = np.empty((2, NUM_LAYERS, LAYER_SIZE), np.float32)
    for c in range(NCORES):
        full[:, c * L : (c + 1) * L, :] = res.results[c]["out"].reshape(
            2, L, LAYER_SIZE
        )
    return full.reshape(2, -1), res


def kernel(**inputs) -> np.ndarray:
    out, _ = _run(inputs, trace=False)
    return out


# revision 21
# speedup vs baseline: 1.1129x; 1.1129x over previous
"""LoRA generator kernel for Trainium2, sharded over 8 NeuronCores by layer.

Reference computation (see problem):
  pe = (condition @ W_proj + b_proj)                        (B=2, 224, 512)
  A  = (gelu(pe@WA1+bA1) @ WA2 + bA2) -> (B, L, 7, 16, 64)
  Bm = (gelu(pe@WB1+bB1) @ WB2 + bB2) -> (B, L, 7, 64, 16)
  out per (b, layer): concat over t of [tile_cols(A)*scA (16 x in_d),
                                        tile_rows(B)*scB (out_d x 16)]

Each core handles 4 layers (28 of the 224 projections). The big costs are
streaming its W_proj slice (11MB in bf16) and writing its 36.8MB output
slice. Pipeline: 4 rounds, one layer each — round r loads layer r's W_proj
columns (one 2.75MB DMA), computes pe, decodes, and drains the layer's
output while round r+1 loads.

Output path is built for large DMA descriptors:
  A pieces: decoder rows are scattered to partition (rank*8+slot), the
    64-float base chunk is expanded to 4096 floats by 6 DVE doubling
    copies, and each piece is written with 16KB descriptors (one per
    rank-row; in_d=11008 uses a step-0 repeat plus tail).
  B pieces: each piece is one 4KB block repeated out_d/64 times; the block
    is placed on an engine-balanced partition with 4 copies materialized
    (16KB), then written with 16KB descriptors using a step-0 repeat dim.
"""
import sys

sys.path.insert(0, "/opt/trn_rl_repo")

import numpy as np
import ml_dtypes

import concourse.bass as bass
import concourse.bacc as bacc
import concourse.mybir as mybir
import concourse.tile as tile
from concourse.bass_utils import run_bass_kernel_spmd

F32 = mybir.dt.float32
BF16 = mybir.dt.bfloat16
ACT_FN = mybir.ActivationFunctionType.Gelu  # sim override hook
SCATTER_ENG = lambda nc: nc.sync  # probe hook
NPBF16 = ml_dtypes.bfloat16

NCORES = 8
NUM_LAYERS = 32
RANK = 16
PED = 512
EMB = 384
T = 7
L = NUM_LAYERS // NCORES          # 4 layers per core
LT = L * T                        # 28 projections per core
ROWS = 2 * LT                     # 56 rows (b, l, t); row = (l*7+t)*2 + b
WP_COLS = LT * PED                # 14336
RPL = 2 * T                       # 14 rows per layer

IN_DS = [4096, 4096, 4096, 4096, 4096, 4096, 11008]
OUT_DS = [4096, 1024, 1024, 4096, 11008, 11008, 4096]
A_SIZES = [16 * d for d in IN_DS]
B_SIZES = [16 * d for d in OUT_DS]
LAYER_SIZE = sum(A_SIZES) + sum(B_SIZES)   # 1150976
OFF_A = []
OFF_B = []
_o = 0
for _t in range(T):
    OFF_A.append(_o)
    _o += A_SIZES[_t]
    OFF_B.append(_o)
    _o += B_SIZES[_t]
OUT_SZ = 2 * L * LAYER_SIZE

RCOLS = T * PED                   # 3584 W_proj columns per round (1 layer)

PB_L = [0, 32, 64, 64]           # partition base per layer (engine ops need 0/32/64)
ACOL = [0, 0, 0, 1024]           # oa column offset per layer
BCOL = [0, 0, 0, 1024]           # ob / scaled-bias column offset per layer

# slot -> (t, b) within a group; even groups hold rows 0-6 of the layer,
# odd groups rows 7-13, where row = 2*t + b.
SLOT_TB = [
    [(0, 0), (0, 1), (1, 0), (1, 1), (2, 0), (2, 1), (3, 0)],
    [(3, 1), (4, 0), (4, 1), (5, 0), (5, 1), (6, 0), (6, 1)],
]

# B piece placement: each (t, b) piece's 16KB block is replicated on 4
# partitions (stride 8) mapping to 4 distinct SDMA engines; the map spreads
# the per-layer byte load evenly over all 16 engines.
B_PART = {
    (0, 0): 32, (0, 1): 36,
    (1, 0): 65, (1, 1): 69,
    (2, 0): 33, (2, 1): 37,
    (3, 0): 96, (3, 1): 100,
    (4, 0): 0, (4, 1): 4,
    (5, 0): 64, (5, 1): 68,
    (6, 0): 1, (6, 1): 5,
}
NB4 = [d // 256 for d in OUT_DS]   # 16KB-descriptor repeats per piece


def _gbase(g):
    """First partition of row-group g (7 rows each, g = 2*l + parity)."""
    return PB_L[g // 2] + 7 * (g % 2)


def _build_nc():
    nc = bacc.Bacc(None, target_bir_lowering=False, debug=False)

    cond = nc.declare_dram_parameter("cond", [128, 6], BF16, isOutput=False)
    wp = nc.declare_dram_parameter("wp", [EMB, WP_COLS], BF16, isOutput=False)
    bpt = nc.declare_dram_parameter("bpt", [128, 4 * LT], F32, isOutput=False)
    wa1 = nc.declare_dram_parameter("wa1", [128, 1024], BF16, isOutput=False)
    wb1 = nc.declare_dram_parameter("wb1", [128, 1024], BF16, isOutput=False)
    wa2 = nc.declare_dram_parameter("wa2", [128, 2048], BF16, isOutput=False)
    wb2 = nc.declare_dram_parameter("wb2", [128, 2048], BF16, isOutput=False)
    ba1 = nc.declare_dram_parameter("ba1", [128, 2], F32, isOutput=False)
    bb1 = nc.declare_dram_parameter("bb1", [128, 2], F32, isOutput=False)
    sca = nc.declare_dram_parameter("sca", [128, ROWS], BF16, isOutput=False)
    scb = nc.declare_dram_parameter("scb", [128, ROWS], BF16, isOutput=False)
    sba2 = nc.declare_dram_parameter("sba2", [128, 2048], F32, isOutput=False)
    sbb2 = nc.declare_dram_parameter("sbb2", [128, 2048], F32, isOutput=False)
    ident = nc.declare_dram_parameter("ident", [128, 2], F32, isOutput=False)
    out = nc.declare_dram_parameter("out", [OUT_SZ], F32, isOutput=True)

    with tile.TileContext(nc) as tc:
        with (
            tc.tile_pool(name="const", bufs=1) as cpool,
            tc.tile_pool(name="wp", bufs=2) as wpool,
            tc.tile_pool(name="work", bufs=1) as wkpool,
            tc.tile_pool(name="pe2", bufs=2) as pe2pool,
            tc.tile_pool(name="ps", bufs=1, space="PSUM") as ps,
        ):
            cond_sb = cpool.tile([128, 6], BF16)
            nc.gpsimd.dma_start(cond_sb[:], cond[:])
            bpt_sb = cpool.tile([128, 4 * LT], F32)
            nc.sync.dma_start(bpt_sb[:], bpt[:])
            wa1_sb = cpool.tile([128, 1024], BF16)
            nc.sync.dma_start(wa1_sb[:], wa1[:])
            wb1_sb = cpool.tile([128, 1024], BF16)
            nc.sync.dma_start(wb1_sb[:], wb1[:])
            wa2_sb = cpool.tile([128, 2048], BF16)
            nc.sync.dma_start(wa2_sb[:], wa2[:])
            wb2_sb = cpool.tile([128, 2048], BF16)
            nc.sync.dma_start(wb2_sb[:], wb2[:])
            ba1_sb = cpool.tile([128, 2], F32)
            nc.sync.dma_start(ba1_sb[:], ba1[:])
            bb1_sb = cpool.tile([128, 2], F32)
            nc.sync.dma_start(bb1_sb[:], bb1[:])
            sca_sb = cpool.tile([128, ROWS], BF16)
            nc.sync.dma_start(sca_sb[:], sca[:])
            scb_sb = cpool.tile([128, ROWS], BF16)
            nc.sync.dma_start(scb_sb[:], scb[:])
            sba2_sb = cpool.tile([128, 2048], F32)
            nc.scalar.dma_start(sba2_sb[:], sba2[:])
            sbb2_sb = cpool.tile([128, 2048], F32)
            nc.scalar.dma_start(sbb2_sb[:], sbb2[:])
            ident_sb = cpool.tile([128, 2], F32)
            nc.sync.dma_start(ident_sb[:], ident[:])

            # long-lived work tiles
            pe_sb = [
                wkpool.tile([128, ROWS], BF16, tag=f"pe_sb{mc}", name=f"pe_sb{mc}")
                for mc in range(4)
            ]
            oa = wkpool.tile([128, 2048], F32)     # decoder A out
            ob_sb = wkpool.tile([128, 2048], F32)  # decoder B out
            pa = oa[:, :].ap[0][0]
            pob = ob_sb[:, :].ap[0][0]
            oa_t = oa[:, :].tensor
            ob_t = ob_sb[:, :].tensor
            # expansion buffers, rotated manually (layer l+k reuses after
            # the piece DMAs of layer l drained; Tile tracks the WAR deps)
            aexp_bufs = [
                wkpool.tile([128, 4096], F32, tag=f"aexpb{i}", name=f"aexpb{i}")
                for i in range(3)
            ]
            bexp_bufs = [
                wkpool.tile([128, 4096], F32, tag=f"bexpb{i}", name=f"bexpb{i}")
                for i in range(2)
            ]
            anchor_sb = wkpool.tile([128, 8], F32, tag="anchor", name="anchor_sb")
            bstage = wkpool.tile([128, 1024], F32, tag="bstage", name="bstage")

            def decode_layer(l):
                """Decoder MLPs + expansion + piece DMAs for layer l."""
                c0 = RPL * l              # first row / pe_sb column of the layer
                pb = PB_L[l]              # partition base (0/32/64)
                acol, bcol = ACOL[l], BCOL[l]
                for dec, (w1_sb, b1_sb, w2_sb, sc_sb, sb2_sb) in enumerate(
                    [
                        (wa1_sb, ba1_sb, wa2_sb, sca_sb, sba2_sb),
                        (wb1_sb, bb1_sb, wb2_sb, scb_sb, sbb2_sb),
                    ]
                ):
                    h_sb = []
                    for mc in range(2):
                        hp = ps.tile([128, RPL], F32, tag=f"h{mc}", name=f"hp{mc}")
                        for kc in range(4):
                            nc.tensor.matmul(
                                hp[:],
                                w1_sb[:, kc * 256 + mc * 128 : kc * 256 + (mc + 1) * 128],
                                pe_sb[kc][:, c0 : c0 + RPL],
                                start=(kc == 0),
                                stop=(kc == 3),
                            )
                        hs = wkpool.tile(
                            [128, RPL], BF16, tag=f"h_sb{dec}{mc}", name=f"hs{dec}{mc}"
                        )
                        nc.scalar.activation(
                            hs[:], hp[:], ACT_FN,
                            bias=b1_sb[:, mc : mc + 1],
                        )
                        nc.vector.tensor_mul(hs[:], hs[:], sc_sb[:, c0 : c0 + RPL])
                        h_sb.append(hs)
                    for nh in range(2):
                        op = ps.tile([128, 512], F32, tag=f"o{nh}", name=f"op{nh}")
                        for kc in range(2):
                            nc.tensor.matmul(
                                op[pb : pb + RPL, :],
                                h_sb[kc][:],
                                w2_sb[:, kc * 1024 + nh * 512 : kc * 1024 + (nh + 1) * 512],
                                start=(kc == 0),
                                stop=(kc == 1),
                            )
                        tgt = oa if dec == 0 else ob_sb
                        coff = acol if dec == 0 else bcol
                        nc.vector.tensor_add(
                            tgt[pb : pb + RPL, coff + nh * 512 : coff + (nh + 1) * 512],
                            op[pb : pb + RPL, :],
                            sb2_sb[pb : pb + RPL, bcol + nh * 512 : bcol + (nh + 1) * 512],
                        )

                # ---- A pieces ----
                # stage 1 for both groups: scatter + first doubling (sync
                # queue, FIFO-ordered behind the previous user of the buffer)
                for g in (2 * l, 2 * l + 1):
                    gb = _gbase(g)
                    aexp = aexp_bufs[g % 3]
                    aexp_t = aexp[:, :].tensor
                    pax = aexp[:, :].ap[0][0]
                    # scatter: aexp[8r+slot, 0:64] = oa[gb+slot, acol+64r:+64]
                    # slot 7 duplicates slot 0 so every partition is
                    # initialized for the doubling copies (lanes p%8==7 are
                    # never read by the piece DMAs)
                    for s in range(8):
                        src = oa[gb + min(s, 6) : gb + min(s, 6) + 1,
                                 acol : acol + 1024]
                        dst = bass.AP(
                            aexp_t, s * pax, [[8 * pax, 16], [1, 64]]
                        )
                        nc.sync.dma_start(dst, src)
                    # first doubling on the sync queue with tracked slice APs
                    # (orders the DVE chain behind the raw-AP scatters)
                    nc.sync.dma_start(aexp[:, 64:128], aexp[:, 0:64])
                # stage 2: DVE doubling chains (128 -> 4096)
                for g in (2 * l, 2 * l + 1):
                    aexp = aexp_bufs[g % 3]
                    w = 128
                    while w < 4096:
                        nc.vector.tensor_copy(
                            aexp[:, w : 2 * w], aexp[:, 0:w]
                        )
                        w *= 2
                # stage 3: anchor (tracked read of the DVE chain tail) then
                # piece DMAs (FIFO behind the anchor on the sync queue)
                for g in (2 * l, 2 * l + 1):
                    gb = _gbase(g)
                    aexp = aexp_bufs[g % 3]
                    aexp_t = aexp[:, :].tensor
                    pax = aexp[:, :].ap[0][0]
                    nc.sync.dma_start(
                        anchor_sb[:, g % 8 : g % 8 + 1], aexp[:, 4095:4096]
                    )
                    for s in range(7):
                        t, b = SLOT_TB[g % 2][s]
                        in_d = IN_DS[t]
                        base = (b * L + l) * LAYER_SIZE + OFF_A[t]
                        if in_d == 4096:
                            dstp = bass.AP(out, base, [[4096, 16], [1, 4096]])
                            srcp = bass.AP(
                                aexp_t, s * pax, [[8 * pax, 16], [1, 4096]]
                            )
                            nc.sync.dma_start(dstp, srcp)
                        else:  # 11008 = 2*4096 + 2816
                            dstp = bass.AP(
                                out, base, [[in_d, 16], [4096, 2], [1, 4096]]
                            )
                            srcp = bass.AP(
                                aexp_t, s * pax, [[8 * pax, 16], [0, 2], [1, 4096]]
                            )
                            nc.sync.dma_start(dstp, srcp)
                            dstp = bass.AP(
                                out, base + 8192, [[in_d, 16], [1, 2816]]
                            )
                            srcp = bass.AP(
                                aexp_t, s * pax, [[8 * pax, 16], [1, 2816]]
                            )
                            nc.sync.dma_start(dstp, srcp)

                # ---- B pieces ----
                # tracked staging read of ob (gives the scalar queue a proper
                # dependency on the DVE adds; everything after is FIFO)
                nc.scalar.dma_start(
                    bstage[0:RPL, :], ob_sb[pb : pb + RPL, bcol : bcol + 1024]
                )
                bst_t = bstage[:, :].tensor
                pst = bstage[:, :].ap[0][0]
                for t in range(T):
                    bexp = bexp_bufs[t % 2]
                    bexp_t = bexp[:, :].tensor
                    pbx = bexp[:, :].ap[0][0]
                    for b in range(2):
                        p0 = B_PART[(t, b)]
                        # replicate the 4KB block x4 onto 4 partitions
                        # (stride 8 -> 4 distinct engines), 16KB each
                        src = bass.AP(
                            bst_t, (2 * t + b) * pst,
                            [[pst, 1], [0, 16], [1, 1024]],
                        )
                        dst = bass.AP(
                            bexp_t, p0 * pbx,
                            [[8 * pbx, 4], [1024, 4], [1, 1024]],
                        )
                        nc.scalar.dma_start(dst, src)
                for t in range(T):
                    bexp = bexp_bufs[t % 2]
                    bexp_t = bexp[:, :].tensor
                    pbx = bexp[:, :].ap[0][0]
                    nb4 = NB4[t]
                    nq, nr = nb4 // 4, nb4 % 4
                    for b in range(2):
                        p0 = B_PART[(t, b)]
                        base = (b * L + l) * LAYER_SIZE + OFF_B[t]
                        dstp = bass.AP(
                            out, base,
                            [[nq * 4096, 4], [4096, nq], [1, 4096]],
                        )
                        srcp = bass.AP(
                            bexp_t, p0 * pbx,
                            [[8 * pbx, 4], [0, nq], [1, 4096]],
                        )
                        nc.scalar.dma_start(dstp, srcp)
                        if nr:
                            dstp = bass.AP(
                                out, base + 4 * nq * 4096,
                                [[4096, nr], [1, 4096]],
                            )
                            srcp = bass.AP(
                                bexp_t, p0 * pbx,
                                [[8 * pbx, 1], [0, nr], [1, 4096]],
                            )
                            nc.scalar.dma_start(dstp, srcp)

            # ---- main pipeline: one layer per round ----
            for rd in range(L):
                wp_t = wpool.tile([128, 3 * RCOLS], BF16, tag="wp", name=f"wp{rd}")
                pwt = wp_t[:, :].ap[0][0]
                wp_src = bass.AP(
                    wp, rd * RCOLS, [[WP_COLS, 128], [128 * WP_COLS, 3], [1, RCOLS]]
                )
                wp_dst = bass.AP(
                    wp_t[:, :].tensor, 0, [[pwt, 128], [RCOLS, 3], [1, RCOLS]]
                )
                nc.gpsimd.dma_start(wp_dst, wp_src)
                pe2_sb = pe2pool.tile([2, RCOLS], F32, tag="pe2sb", name="pe2_sb")
                for ltl in range(T):
                    p2 = ps.tile([2, PED], F32, tag=f"p2{ltl % 2}", name="pe2_ps")
                    for kc in range(3):
                        nc.tensor.matmul(
                            p2[:],
                            cond_sb[:, kc * 2 : kc * 2 + 2],
                            wp_t[:, kc * RCOLS + ltl * PED : kc * RCOLS + (ltl + 1) * PED],
                            start=(kc == 0),
                            stop=(kc == 2),
                        )
                    nc.vector.tensor_copy(pe2_sb[:, ltl * PED : (ltl + 1) * PED], p2[:])
                for ltl in range(T):
                    lt = rd * T + ltl
                    for mc in range(4):
                        tr = ps.tile([128, 2], F32, tag=f"tr{mc % 2}", name="tr_ps")
                        nc.tensor.transpose(
                            tr[:],
                            pe2_sb[:, ltl * PED + mc * 128 : ltl * PED + (mc + 1) * 128],
                            ident_sb[0:2, 0:2],
                        )
                        # pe_T with b_proj bias (per-partition, same for both b)
                        nc.vector.tensor_scalar_add(
                            pe_sb[mc][:, 2 * lt : 2 * lt + 2],
                            tr[:],
                            bpt_sb[:, mc * LT + lt : mc * LT + lt + 1],
                        )
                decode_layer(rd)

    nc.finalize()
    return nc


_NC = None


def _get_nc():
    global _NC
    if _NC is None:
        _NC = _build_nc()
    return _NC


def _marshal(inputs):
    """Build the per-core input maps from full inputs."""
    condition = np.asarray(inputs["condition"], np.float32)
    W_proj = np.asarray(inputs["W_proj"], np.float32)
    b_proj = np.asarray(inputs["b_proj"], np.float32)
    WA1 = np.asarray(inputs["WA1"], np.float32)
    bA1 = np.asarray(inputs["bA1"], np.float32)
    WA2 = np.asarray(inputs["WA2"], np.float32)
    bA2 = np.asarray(inputs["bA2"], np.float32)
    WB1 = np.asarray(inputs["WB1"], np.float32)
    bB1 = np.asarray(inputs["bB1"], np.float32)
    WB2 = np.asarray(inputs["WB2"], np.float32)
    bB2 = np.asarray(inputs["bB2"], np.float32)
    scales = np.asarray(inputs["scales"], np.float32)

    cond_arr = np.zeros((128, 6), np.float32)
    for kc in range(3):
        cond_arr[:, kc * 2 : kc * 2 + 2] = condition[:, kc * 128 : (kc + 1) * 128].T
    cond_arr = cond_arr.astype(NPBF16)
    wa1_arr = np.zeros((128, 1024), np.float32)
    wb1_arr = np.zeros((128, 1024), np.float32)
    for kc in range(4):
        wa1_arr[:, kc * 256 : (kc + 1) * 256] = WA1[kc * 128 : (kc + 1) * 128, :]
        wb1_arr[:, kc * 256 : (kc + 1) * 256] = WB1[kc * 128 : (kc + 1) * 128, :]
    wa2_arr = np.zeros((128, 2048), np.float32)
    wb2_arr = np.zeros((128, 2048), np.float32)
    for kc in range(2):
        wa2_arr[:, kc * 1024 : (kc + 1) * 1024] = WA2[kc * 128 : (kc + 1) * 128, :]
        wb2_arr[:, kc * 1024 : (kc + 1) * 1024] = WB2[kc * 128 : (kc + 1) * 128, :]
    wa1_arr = wa1_arr.astype(NPBF16)
    wb1_arr = wb1_arr.astype(NPBF16)
    wa2_arr = wa2_arr.astype(NPBF16)
    wb2_arr = wb2_arr.astype(NPBF16)
    ba1_arr = np.ascontiguousarray(bA1.reshape(2, 128).T)
    bb1_arr = np.ascontiguousarray(bB1.reshape(2, 128).T)
    ident_arr = np.zeros((128, 2), np.float32)
    ident_arr[0, 0] = 1.0
    ident_arr[1, 1] = 1.0

    in_maps = []
    for c in range(NCORES):
        lt0 = c * LT
        wp_c = np.ascontiguousarray(
            W_proj[:, lt0 * PED : (lt0 + LT) * PED]
        ).astype(NPBF16)
        bp_c = b_proj[lt0 * PED : (lt0 + LT) * PED].reshape(LT, 4, 128)
        bpt_arr = np.zeros((128, 4 * LT), np.float32)
        for lt in range(LT):
            for mc in range(4):
                bpt_arr[:, mc * LT + lt] = bp_c[lt, mc, :]
        sca_row = np.zeros(ROWS, np.float32)
        scb_row = np.zeros(ROWS, np.float32)
        for row in range(ROWS):
            lt = row // 2
            sca_row[row] = scales[lt0 + lt, 0]
            scb_row[row] = scales[lt0 + lt, 1]
        sca_arr = np.broadcast_to(sca_row[None, :], (128, ROWS)).astype(NPBF16)
        scb_arr = np.broadcast_to(scb_row[None, :], (128, ROWS)).astype(NPBF16)
        sba2_arr = np.zeros((128, 2048), np.float32)
        sbb2_arr = np.zeros((128, 2048), np.float32)
        for row in range(ROWS):
            l = row // RPL
            p = PB_L[l] + (row % RPL)
            blk = BCOL[l]
            sba2_arr[p, blk : blk + 1024] = sca_row[row] * bA2
            sbb2_arr[p, blk : blk + 1024] = scb_row[row] * bB2
        in_maps.append(
            {
                "cond": cond_arr,
                "wp": wp_c,
                "bpt": bpt_arr,
                "wa1": wa1_arr,
                "wb1": wb1_arr,
                "wa2": wa2_arr,
                "wb2": wb2_arr,
                "ba1": ba1_arr,
                "bb1": bb1_arr,
                "sca": sca_arr,
                "scb": scb_arr,
                "sba2": sba2_arr,
                "sbb2": sbb2_arr,
                "ident": ident_arr,
            }
        )
    return in_maps


def _ensure_ntff_hook():
    """Register the axon NTFF profile hook if the boot didn't (module was
    missing at boot time)."""
    import types

    ah = sys.modules.get("antenv.axon_hooks")
    if ah is None:
        ah = types.ModuleType("antenv.axon_hooks")
        ah._hook = None

        def _set(h, _m=ah):
            _m._hook = h

        def _get(_m=ah):
            return _m._hook

        ah.set_axon_ntff_profile_hook = _set
        ah.get_axon_ntff_profile_hook = _get
        sys.modules["antenv.axon_hooks"] = ah
        import antenv

        antenv.axon_hooks = ah
    if ah.get_axon_ntff_profile_hook() is None:
        if "/root/.axon_site" not in sys.path:
            sys.path.insert(0, "/root/.axon_site")
        from trn_agent_boot.trn_boot import _ntff_profile_via_ctypes

        hook = _ntff_profile_via_ctypes("/opt/axon/libaxon_pjrt.so")
        if hook is not None:
            ah.set_axon_ntff_profile_hook(hook)


def _run(inputs, trace=False):
    if trace:
        _ensure_ntff_hook()
    nc = _get_nc()
    in_maps = _marshal(inputs)
    res = run_bass_kernel_spmd(nc, in_maps, list(range(NCORES)), trace=trace)
    full = np.empty((2, NUM_LAYERS, LAYER_SIZE), np.float32)
    for c in range(NCORES):
        full[:, c * L : (c + 1) * L, :] = res.results[c]["out"].reshape(
            2, L, LAYER_SIZE
        )
    return full.reshape(2, -1), res


def kernel(**inputs) -> np.ndarray:
    out, _ = _run(inputs, trace=False)
    return out


# revision 24
# speedup vs baseline: 1.3160x; 1.1825x over previous
"""LoRA generator kernel for Trainium2, sharded over 8 NeuronCores by layer.

Reference computation (see problem):
  pe = (condition @ W_proj + b_proj)                        (B=2, 224, 512)
  A  = (gelu(pe@WA1+bA1) @ WA2 + bA2) -> (B, L, 7, 16, 64)
  Bm = (gelu(pe@WB1+bB1) @ WB2 + bB2) -> (B, L, 7, 64, 16)
  out per (b, layer): concat over t of [tile_cols(A)*scA (16 x in_d),
                                        tile_rows(B)*scB (out_d x 16)]

Each core handles 4 layers (28 of the 224 projections). The big costs are
streaming its W_proj slice (11MB in bf16) and writing its 36.8MB output
slice. Pipeline: 4 rounds, one layer each — round r loads layer r's W_proj
columns (one 2.75MB DMA), computes pe, decodes, and drains the layer's
output while round r+1 loads.

Output path is built for large DMA descriptors:
  A pieces: decoder rows are scattered to partition (rank*8+slot), the
    64-float base chunk is expanded to 4096 floats by 6 DVE doubling
    copies, and each piece is written with 16KB descriptors (one per
    rank-row; in_d=11008 uses a step-0 repeat plus tail).
  B pieces: each piece is one 4KB block repeated out_d/64 times; the block
    is placed on an engine-balanced partition with 4 copies materialized
    (16KB), then written with 16KB descriptors using a step-0 repeat dim.
"""
import sys

sys.path.insert(0, "/opt/trn_rl_repo")

import numpy as np
import ml_dtypes

import concourse.bass as bass
import concourse.bacc as bacc
import concourse.mybir as mybir
import concourse.tile as tile
from concourse.bass_utils import run_bass_kernel_spmd

F32 = mybir.dt.float32
BF16 = mybir.dt.bfloat16
ACT_FN = mybir.ActivationFunctionType.Gelu  # sim override hook
SCATTER_ENG = lambda nc: nc.sync  # probe hook
NPBF16 = ml_dtypes.bfloat16

NCORES = 8
NUM_LAYERS = 32
RANK = 16
PED = 512
EMB = 384
T = 7
L = NUM_LAYERS // NCORES          # 4 layers per core
LT = L * T                        # 28 projections per core
ROWS = 2 * LT                     # 56 rows (b, l, t); row = (l*7+t)*2 + b
WP_COLS = LT * PED                # 14336
RPL = 2 * T                       # 14 rows per layer

IN_DS = [4096, 4096, 4096, 4096, 4096, 4096, 11008]
OUT_DS = [4096, 1024, 1024, 4096, 11008, 11008, 4096]
A_SIZES = [16 * d for d in IN_DS]
B_SIZES = [16 * d for d in OUT_DS]
LAYER_SIZE = sum(A_SIZES) + sum(B_SIZES)   # 1150976
OFF_A = []
OFF_B = []
_o = 0
for _t in range(T):
    OFF_A.append(_o)
    _o += A_SIZES[_t]
    OFF_B.append(_o)
    _o += B_SIZES[_t]
OUT_SZ = 2 * L * LAYER_SIZE

RCOLS = T * PED                   # 3584 W_proj columns per round (1 layer)

PB_L = [0, 32, 64, 64]           # partition base per layer (engine ops need 0/32/64)
ACOL = [0, 0, 0, 1024]           # oa column offset per layer
BCOL = [0, 0, 0, 1024]           # ob / scaled-bias column offset per layer

# slot -> (t, b) within a group; even groups hold rows 0-6 of the layer,
# odd groups rows 7-13, where row = 2*t + b.
SLOT_TB = [
    [(0, 0), (0, 1), (1, 0), (1, 1), (2, 0), (2, 1), (3, 0)],
    [(3, 1), (4, 0), (4, 1), (5, 0), (5, 1), (6, 0), (6, 1)],
]

# B piece layout: piece i = 2t+b owns partition group 16*(i%8)..+16 and
# column (i//8)*1024 of the layer's bexp buffer; its 4KB block is replicated
# on all 16 group partitions so the out DMA's descriptors (engine = partition
# iteration index mod 16) spread over all 16 SDMA engines.
NB = [d // 64 for d in OUT_DS]     # 4KB chunks per piece


def _gbase(g):
    """First partition of row-group g (7 rows each, g = 2*l + parity)."""
    return PB_L[g // 2] + 7 * (g % 2)


def _build_nc():
    nc = bacc.Bacc(None, target_bir_lowering=False, debug=False)

    cond = nc.declare_dram_parameter("cond", [128, 6], BF16, isOutput=False)
    wp = nc.declare_dram_parameter("wp", [EMB, WP_COLS], BF16, isOutput=False)
    bpt = nc.declare_dram_parameter("bpt", [128, 4 * LT], F32, isOutput=False)
    wa1 = nc.declare_dram_parameter("wa1", [128, 1024], BF16, isOutput=False)
    wb1 = nc.declare_dram_parameter("wb1", [128, 1024], BF16, isOutput=False)
    wa2 = nc.declare_dram_parameter("wa2", [128, 2048], BF16, isOutput=False)
    wb2 = nc.declare_dram_parameter("wb2", [128, 2048], BF16, isOutput=False)
    ba1 = nc.declare_dram_parameter("ba1", [128, 2], F32, isOutput=False)
    bb1 = nc.declare_dram_parameter("bb1", [128, 2], F32, isOutput=False)
    sca = nc.declare_dram_parameter("sca", [128, ROWS], BF16, isOutput=False)
    scb = nc.declare_dram_parameter("scb", [128, ROWS], BF16, isOutput=False)
    sba2 = nc.declare_dram_parameter("sba2", [128, 2048], F32, isOutput=False)
    sbb2 = nc.declare_dram_parameter("sbb2", [128, 2048], F32, isOutput=False)
    ident = nc.declare_dram_parameter("ident", [128, 2], F32, isOutput=False)
    out = nc.declare_dram_parameter("out", [OUT_SZ], F32, isOutput=True)

    with tile.TileContext(nc) as tc:
        with (
            tc.tile_pool(name="const", bufs=1) as cpool,
            tc.tile_pool(name="wp", bufs=2) as wpool,
            tc.tile_pool(name="work", bufs=1) as wkpool,
            tc.tile_pool(name="pe2", bufs=2) as pe2pool,
            tc.tile_pool(name="ps", bufs=1, space="PSUM") as ps,
        ):
            cond_sb = cpool.tile([128, 6], BF16)
            nc.gpsimd.dma_start(cond_sb[:], cond[:])
            bpt_sb = cpool.tile([128, 4 * LT], F32)
            nc.sync.dma_start(bpt_sb[:], bpt[:])
            wa1_sb = cpool.tile([128, 1024], BF16)
            nc.sync.dma_start(wa1_sb[:], wa1[:])
            wb1_sb = cpool.tile([128, 1024], BF16)
            nc.sync.dma_start(wb1_sb[:], wb1[:])
            wa2_sb = cpool.tile([128, 2048], BF16)
            nc.sync.dma_start(wa2_sb[:], wa2[:])
            wb2_sb = cpool.tile([128, 2048], BF16)
            nc.sync.dma_start(wb2_sb[:], wb2[:])
            ba1_sb = cpool.tile([128, 2], F32)
            nc.sync.dma_start(ba1_sb[:], ba1[:])
            bb1_sb = cpool.tile([128, 2], F32)
            nc.sync.dma_start(bb1_sb[:], bb1[:])
            sca_sb = cpool.tile([128, ROWS], BF16)
            nc.sync.dma_start(sca_sb[:], sca[:])
            scb_sb = cpool.tile([128, ROWS], BF16)
            nc.sync.dma_start(scb_sb[:], scb[:])
            sba2_sb = cpool.tile([128, 2048], F32)
            nc.scalar.dma_start(sba2_sb[:], sba2[:])
            sbb2_sb = cpool.tile([128, 2048], F32)
            nc.scalar.dma_start(sbb2_sb[:], sbb2[:])
            ident_sb = cpool.tile([128, 2], F32)
            nc.sync.dma_start(ident_sb[:], ident[:])

            # long-lived work tiles
            pe_sb = [
                wkpool.tile([128, ROWS], BF16, tag=f"pe_sb{mc}", name=f"pe_sb{mc}")
                for mc in range(4)
            ]
            oa = wkpool.tile([128, 2048], F32)     # decoder A out
            ob_sb = wkpool.tile([128, 2048], F32)  # decoder B out
            pa = oa[:, :].ap[0][0]
            pob = ob_sb[:, :].ap[0][0]
            oa_t = oa[:, :].tensor
            ob_t = ob_sb[:, :].tensor
            # expansion buffers, rotated manually (layer l+k reuses after
            # the piece DMAs of layer l drained; Tile tracks the WAR deps)
            aexp_bufs = [
                wkpool.tile([128, 4096], F32, tag=f"aexpb{i}", name=f"aexpb{i}")
                for i in range(3)
            ]
            bexp_bufs = [
                wkpool.tile([128, 2048], F32, tag=f"bexpb{i}", name=f"bexpb{i}")
                for i in range(2)
            ]
            bstage = wkpool.tile([128, 1024], F32, tag="bstage", name="bstage")

            def decode_layer(l):
                """Decoder MLPs + expansion + piece DMAs for layer l."""
                c0 = RPL * l              # first row / pe_sb column of the layer
                pb = PB_L[l]              # partition base (0/32/64)
                acol, bcol = ACOL[l], BCOL[l]
                for dec, (w1_sb, b1_sb, w2_sb, sc_sb, sb2_sb) in enumerate(
                    [
                        (wa1_sb, ba1_sb, wa2_sb, sca_sb, sba2_sb),
                        (wb1_sb, bb1_sb, wb2_sb, scb_sb, sbb2_sb),
                    ]
                ):
                    h_sb = []
                    for mc in range(2):
                        hp = ps.tile([128, RPL], F32, tag=f"h{mc}", name=f"hp{mc}")
                        for kc in range(4):
                            nc.tensor.matmul(
                                hp[:],
                                w1_sb[:, kc * 256 + mc * 128 : kc * 256 + (mc + 1) * 128],
                                pe_sb[kc][:, c0 : c0 + RPL],
                                start=(kc == 0),
                                stop=(kc == 3),
                            )
                        hs = wkpool.tile(
                            [128, RPL], BF16, tag=f"h_sb{dec}{mc}", name=f"hs{dec}{mc}"
                        )
                        nc.scalar.activation(
                            hs[:], hp[:], ACT_FN,
                            bias=b1_sb[:, mc : mc + 1],
                        )
                        nc.vector.tensor_mul(hs[:], hs[:], sc_sb[:, c0 : c0 + RPL])
                        h_sb.append(hs)
                    for nh in range(2):
                        op = ps.tile([128, 512], F32, tag=f"o{nh}", name=f"op{nh}")
                        for kc in range(2):
                            nc.tensor.matmul(
                                op[pb : pb + RPL, :],
                                h_sb[kc][:],
                                w2_sb[:, kc * 1024 + nh * 512 : kc * 1024 + (nh + 1) * 512],
                                start=(kc == 0),
                                stop=(kc == 1),
                            )
                        tgt = oa if dec == 0 else ob_sb
                        coff = acol if dec == 0 else bcol
                        nc.vector.tensor_add(
                            tgt[pb : pb + RPL, coff + nh * 512 : coff + (nh + 1) * 512],
                            op[pb : pb + RPL, :],
                            sb2_sb[pb : pb + RPL, bcol + nh * 512 : bcol + (nh + 1) * 512],
                        )

                # ---- A pieces ----
                # aexp layout: partition 16*slot + rank, so every AP is a
                # contiguous-partition tile slice (tracked deps, and the
                # 16-entry partition iteration spreads over all 16 engines)
                for g in (2 * l, 2 * l + 1):
                    gb = _gbase(g)
                    aexp = aexp_bufs[g % 3]
                    # scatter: aexp[16s+r, 0:64] = oa[gb+s, acol+64r : +64]
                    for s in range(7):
                        nc.sync.dma_start(
                            aexp[16 * s : 16 * s + 16, 0:64],
                            oa[gb + s : gb + s + 1, acol : acol + 1024],
                        )
                # DVE doubling chains 64 -> 4096 on partitions 0:112
                for g in (2 * l, 2 * l + 1):
                    aexp = aexp_bufs[g % 3]
                    w = 64
                    while w < 4096:
                        nc.vector.tensor_copy(
                            aexp[0:112, w : 2 * w], aexp[0:112, 0:w]
                        )
                        w *= 2
                for g in (2 * l, 2 * l + 1):
                    aexp = aexp_bufs[g % 3]
                    aexp_t = aexp[:, :].tensor
                    pax = aexp[:, :].ap[0][0]
                    for s in range(7):
                        t, b = SLOT_TB[g % 2][s]
                        in_d = IN_DS[t]
                        base = (b * L + l) * LAYER_SIZE + OFF_A[t]
                        if in_d == 4096:
                            dstp = bass.AP(out, base, [[4096, 16], [1, 4096]])
                            nc.sync.dma_start(
                                dstp, aexp[16 * s : 16 * s + 16, 0:4096]
                            )
                        else:  # 11008 = 2*4096 + 2816
                            dstp = bass.AP(
                                out, base, [[in_d, 16], [4096, 2], [1, 4096]]
                            )
                            srcp = bass.AP(
                                aexp_t, 16 * s * pax,
                                [[pax, 16], [0, 2], [1, 4096]],
                            )
                            nc.sync.dma_start(dstp, srcp)
                            dstp = bass.AP(
                                out, base + 8192, [[in_d, 16], [1, 2816]]
                            )
                            nc.sync.dma_start(
                                dstp, aexp[16 * s : 16 * s + 16, 0:2816]
                            )

                # ---- B pieces ----
                # tracked staging read of ob, then replicate each piece's 4KB
                # block onto its 16-partition group (slice dst), then write
                # out with 4KB descriptors over all 16 engines
                nc.scalar.dma_start(
                    bstage[0:RPL, :], ob_sb[pb : pb + RPL, bcol : bcol + 1024]
                )
                bst_t = bstage[:, :].tensor
                pst = bstage[:, :].ap[0][0]
                for t in range(T):
                    for b in range(2):
                        i = 2 * t + b
                        gq, col = 16 * (i % 8), (i // 8) * 1024
                        bexp = bexp_bufs[l % 2]
                        src = bass.AP(
                            bst_t, i * pst, [[pst, 1], [0, 16], [1, 1024]]
                        )
                        nc.scalar.dma_start(
                            bexp[gq : gq + 16, col : col + 1024], src
                        )
                for t in range(T):
                    for b in range(2):
                        i = 2 * t + b
                        gq, col = 16 * (i % 8), (i // 8) * 1024
                        bexp = bexp_bufs[l % 2]
                        bexp_t = bexp[:, :].tensor
                        pbx = bexp[:, :].ap[0][0]
                        nb = NB[t]
                        nq, nr = nb // 16, nb % 16
                        base = (b * L + l) * LAYER_SIZE + OFF_B[t]
                        dstp = bass.AP(
                            out, base,
                            [[nq * 1024, 16], [1024, nq], [1, 1024]],
                        )
                        srcp = bass.AP(
                            bexp_t, gq * pbx + col,
                            [[pbx, 16], [0, nq], [1, 1024]],
                        )
                        nc.scalar.dma_start(dstp, srcp)
                        if nr:
                            dstp = bass.AP(
                                out, base + 16 * nq * 1024,
                                [[1024, nr], [1, 1024]],
                            )
                            srcp = bass.AP(
                                bexp_t, gq * pbx + col,
                                [[pbx, nr], [0, 1], [1, 1024]],
                            )
                            nc.scalar.dma_start(dstp, srcp)

            # ---- main pipeline: one layer per round ----
            for rd in range(L):
                wp_t = wpool.tile([128, 3 * RCOLS], BF16, tag="wp", name=f"wp{rd}")
                pwt = wp_t[:, :].ap[0][0]
                wp_src = bass.AP(
                    wp, rd * RCOLS, [[WP_COLS, 128], [128 * WP_COLS, 3], [1, RCOLS]]
                )
                wp_dst = bass.AP(
                    wp_t[:, :].tensor, 0, [[pwt, 128], [RCOLS, 3], [1, RCOLS]]
                )
                nc.gpsimd.dma_start(wp_dst, wp_src)
                pe2_sb = pe2pool.tile([2, RCOLS], F32, tag="pe2sb", name="pe2_sb")
                for ltl in range(T):
                    p2 = ps.tile([2, PED], F32, tag=f"p2{ltl % 2}", name="pe2_ps")
                    for kc in range(3):
                        nc.tensor.matmul(
                            p2[:],
                            cond_sb[:, kc * 2 : kc * 2 + 2],
                            wp_t[:, kc * RCOLS + ltl * PED : kc * RCOLS + (ltl + 1) * PED],
                            start=(kc == 0),
                            stop=(kc == 2),
                        )
                    nc.vector.tensor_copy(pe2_sb[:, ltl * PED : (ltl + 1) * PED], p2[:])
                for ltl in range(T):
                    lt = rd * T + ltl
                    for mc in range(4):
                        tr = ps.tile([128, 2], F32, tag=f"tr{mc % 2}", name="tr_ps")
                        nc.tensor.transpose(
                            tr[:],
                            pe2_sb[:, ltl * PED + mc * 128 : ltl * PED + (mc + 1) * 128],
                            ident_sb[0:2, 0:2],
                        )
                        # pe_T with b_proj bias (per-partition, same for both b)
                        nc.vector.tensor_scalar_add(
                            pe_sb[mc][:, 2 * lt : 2 * lt + 2],
                            tr[:],
                            bpt_sb[:, mc * LT + lt : mc * LT + lt + 1],
                        )
                decode_layer(rd)

    nc.finalize()
    return nc


_NC = None


def _get_nc():
    global _NC
    if _NC is None:
        _NC = _build_nc()
    return _NC


def _marshal(inputs):
    """Build the per-core input maps from full inputs."""
    condition = np.asarray(inputs["condition"], np.float32)
    W_proj = np.asarray(inputs["W_proj"], np.float32)
    b_proj = np.asarray(inputs["b_proj"], np.float32)
    WA1 = np.asarray(inputs["WA1"], np.float32)
    bA1 = np.asarray(inputs["bA1"], np.float32)
    WA2 = np.asarray(inputs["WA2"], np.float32)
    bA2 = np.asarray(inputs["bA2"], np.float32)
    WB1 = np.asarray(inputs["WB1"], np.float32)
    bB1 = np.asarray(inputs["bB1"], np.float32)
    WB2 = np.asarray(inputs["WB2"], np.float32)
    bB2 = np.asarray(inputs["bB2"], np.float32)
    scales = np.asarray(inputs["scales"], np.float32)

    cond_arr = np.zeros((128, 6), np.float32)
    for kc in range(3):
        cond_arr[:, kc * 2 : kc * 2 + 2] = condition[:, kc * 128 : (kc + 1) * 128].T
    cond_arr = cond_arr.astype(NPBF16)
    wa1_arr = np.zeros((128, 1024), np.float32)
    wb1_arr = np.zeros((128, 1024), np.float32)
    for kc in range(4):
        wa1_arr[:, kc * 256 : (kc + 1) * 256] = WA1[kc * 128 : (kc + 1) * 128, :]
        wb1_arr[:, kc * 256 : (kc + 1) * 256] = WB1[kc * 128 : (kc + 1) * 128, :]
    wa2_arr = np.zeros((128, 2048), np.float32)
    wb2_arr = np.zeros((128, 2048), np.float32)
    for kc in range(2):
        wa2_arr[:, kc * 1024 : (kc + 1) * 1024] = WA2[kc * 128 : (kc + 1) * 128, :]
        wb2_arr[:, kc * 1024 : (kc + 1) * 1024] = WB2[kc * 128 : (kc + 1) * 128, :]
    wa1_arr = wa1_arr.astype(NPBF16)
    wb1_arr = wb1_arr.astype(NPBF16)
    wa2_arr = wa2_arr.astype(NPBF16)
    wb2_arr = wb2_arr.astype(NPBF16)
    ba1_arr = np.ascontiguousarray(bA1.reshape(2, 128).T)
    bb1_arr = np.ascontiguousarray(bB1.reshape(2, 128).T)
    ident_arr = np.zeros((128, 2), np.float32)
    ident_arr[0, 0] = 1.0
    ident_arr[1, 1] = 1.0

    in_maps = []
    for c in range(NCORES):
        lt0 = c * LT
        wp_c = np.ascontiguousarray(
            W_proj[:, lt0 * PED : (lt0 + LT) * PED]
        ).astype(NPBF16)
        bp_c = b_proj[lt0 * PED : (lt0 + LT) * PED].reshape(LT, 4, 128)
        bpt_arr = np.zeros((128, 4 * LT), np.float32)
        for lt in range(LT):
            for mc in range(4):
                bpt_arr[:, mc * LT + lt] = bp_c[lt, mc, :]
        sca_row = np.zeros(ROWS, np.float32)
        scb_row = np.zeros(ROWS, np.float32)
        for row in range(ROWS):
            lt = row // 2
            sca_row[row] = scales[lt0 + lt, 0]
            scb_row[row] = scales[lt0 + lt, 1]
        sca_arr = np.broadcast_to(sca_row[None, :], (128, ROWS)).astype(NPBF16)
        scb_arr = np.broadcast_to(scb_row[None, :], (128, ROWS)).astype(NPBF16)
        sba2_arr = np.zeros((128, 2048), np.float32)
        sbb2_arr = np.zeros((128, 2048), np.float32)
        for row in range(ROWS):
            l = row // RPL
            p = PB_L[l] + (row % RPL)
            blk = BCOL[l]
            sba2_arr[p, blk : blk + 1024] = sca_row[row] * bA2
            sbb2_arr[p, blk : blk + 1024] = scb_row[row] * bB2
        in_maps.append(
            {
                "cond": cond_arr,
                "wp": wp_c,
                "bpt": bpt_arr,
                "wa1": wa1_arr,
                "wb1": wb1_arr,
                "wa2": wa2_arr,
                "wb2": wb2_arr,
                "ba1": ba1_arr,
                "bb1": bb1_arr,
                "sca": sca_arr,
                "scb": scb_arr,
                "sba2": sba2_arr,
                "sbb2": sbb2_arr,
                "ident": ident_arr,
            }
        )
    return in_maps


def _ensure_ntff_hook():
    """Register the axon NTFF profile hook if the boot didn't (module was
    missing at boot time)."""
    import types

    ah = sys.modules.get("antenv.axon_hooks")
    if ah is None:
        ah = types.ModuleType("antenv.axon_hooks")
        ah._hook = None

        def _set(h, _m=ah):
            _m._hook = h

        def _get(_m=ah):
            return _m._hook

        ah.set_axon_ntff_profile_hook = _set
        ah.get_axon_ntff_profile_hook = _get
        sys.modules["antenv.axon_hooks"] = ah
        import antenv

        antenv.axon_hooks = ah
    if ah.get_axon_ntff_profile_hook() is None:
        if "/root/.axon_site" not in sys.path:
            sys.path.insert(0, "/root/.axon_site")
        from trn_agent_boot.trn_boot import _ntff_profile_via_ctypes

        hook = _ntff_profile_via_ctypes("/opt/axon/libaxon_pjrt.so")
        if hook is not None:
            ah.set_axon_ntff_profile_hook(hook)


def _run(inputs, trace=False):
    if trace:
        _ensure_ntff_hook()
    nc = _get_nc()
    in_maps = _marshal(inputs)
    res = run_bass_kernel_spmd(nc, in_maps, list(range(NCORES)), trace=trace)
    full = np.empty((2, NUM_LAYERS, LAYER_SIZE), np.float32)
    for c in range(NCORES):
        full[:, c * L : (c + 1) * L, :] = res.results[c]["out"].reshape(
            2, L, LAYER_SIZE
        )
    return full.reshape(2, -1), res


def kernel(**inputs) -> np.ndarray:
    out, _ = _run(inputs, trace=False)
    return out
